# revision 19
# baseline (speedup 1.0000x reference)
"""Trainium2 Bass kernel for NeuralDisCoCirc forward pass.

Problem: L=8 sequential layers; each layer, per sample b:
    z = h @ W[l,b] + bias[l,b];  h = where(mask[l,b], relu(z), z)
Shapes: x [16,1024] f32, weights [8,16,1024,1024] f32,
        biases/masks [8,16,1024].

Strategy (data-parallel over batch, 2 samples per core, 8 cores):
  - Weights stream as bf16 (host cast): 32 MB per core instead of 64,
    halving the HBM roofline.  rel err ~4e-3 vs the 2e-2 gate.
  - Host lays each core's weight shard out as [t=l*2+b, p, ki*1024+j]
    with W row i = ki*128+p (chunk-major), so per-layer weight loads are
    contiguous [128 x 4KB] DMAs (1 MB halves) on both HWDGE rings with
    4-deep prefetch.  Last tile is re-laid jb-major and streamed as 8
    blocks so its matmuls overlap the DMA tail.
  - h lives in chunk-major column layout ([p, c], element i = c*128+p)
    and is the matmul stationary operand.  Per layer: 2 PSUM groups of
    8 accumulating matmuls (h chunk [128,1] stationary, W chunk
    [128,512] bf16 moving) produce z as a [1,1024] PSUM row.
  - The row->column transpose runs on the PE (8 transposes of [1,128]
    via identity, ~0.1us each) instead of a SWDGE scatter: the scatter's
    ~4-6us software-DMA latency was the old bottleneck (PE idled 4.7us
    per tile waiting for h; DMA rings stalled on slot release).
    Pipelined order per tile t:
      [MM(t) jb0 x8][T(t-1) jb1 x4][MM(t) jb1 x8][T(t) jb0 x4]
    so every transpose has its PSUM->SBUF copy latency hidden behind a
    matmul group, and h(t-1) is ready one full tile before it is needed.
  - DVE does the two PSUM-row->SBUF copies (cast to bf16) and the
    bias + masked-relu (h = zb - mask*min(zb,0)) on [128,8] columns.
  - Final layer skips the column layout: bias + masked relu run in ROW
    layout per jb half and the output ships row-contiguous.
"""

import numpy as np

import concourse.bass as bass
import concourse.mybir as mybir
from concourse import bacc
from concourse.tile import TileContext
from concourse.bass_utils import run_bass_kernel_spmd

L = 8          # layers
B = 16         # full batch
D = 1024       # width
NCORES = 8
BC = B // NCORES   # samples per core (2)
NT = L * BC        # (layer, sample) tiles per core (16)
KI = D // 128      # 8 chunks of 128 along the contraction dim
P = 128

F32 = mybir.dt.float32
BF16 = mybir.dt.bfloat16

WDT = BF16   # weight / h dtype on device
ZDT = BF16   # zrow dtype for the PE transposes (bf16: 1 cyc/row)

_CACHE = {}


def _build() -> bass.Bass:
    nc = bacc.Bacc("TRN2", target_bir_lowering=False, debug=False)
    w = nc.declare_dram_parameter("w", [NT, P, KI * D], WDT, isOutput=False)
    x = nc.declare_dram_parameter("x", [P, BC * KI], WDT, isOutput=False)
    bm = nc.declare_dram_parameter("bm", [P, NT * 2 * KI], F32, isOutput=False)
    # output ships in column layout ([p, b*KI + c], element i = c*128+p);
    # the host transposes it back to row layout.
    out = nc.declare_dram_parameter("out", [P, BC * KI], F32, isOutput=True)

    with TileContext(nc) as tc:
        with (
            tc.tile_pool(name="wp", bufs=6) as wp,  # per-tag: 6 x 1MB x 2 tags
            tc.tile_pool(name="const", bufs=1) as cp,
            tc.tile_pool(name="hrow", bufs=4) as hrp,
            tc.tile_pool(name="hcol", bufs=4) as hcp,
            tc.tile_pool(name="psr", bufs=2, space="PSUM") as psr,
            tc.tile_pool(name="psc", bufs=2, space="PSUM") as psc,
        ):
            # x first on the sync HWDGE ring: tiny (4KB) and it gates the
            # very first matmul, so it must land before the weight stream.
            xt = cp.tile([P, BC * KI], WDT, tag="x")
            nc.sync.dma_start(out=xt, in_=x[:])

            KH = KI // 2  # ki chunks per half-tile
            LAST = NT - 1
            ST = 512      # starter block columns for tile 0 (fast first MM)
            wtiles = {}
            # Issue order: tiles 0..5 (the ungated prefix at bufs=6), then
            # the LAST tile into its own dedicated buffer (so its stream
            # is not serialized by wa/wb slot recycling at the tail),
            # then tiles 6..14.
            issue_order = list(range(6)) + [LAST] + list(range(6, LAST))
            for t in issue_order:
                if t == 0:
                    wa = wp.tile([P, KH * D], WDT, tag="wa")
                    wb = wp.tile([P, KH * D], WDT, tag="wb")
                    nc.sync.dma_start(out=wa[:, :ST], in_=w[t, :, :ST])
                    nc.scalar.dma_start(
                        out=wb[:, :ST], in_=w[t, :, KH * D:KH * D + ST])
                    nc.sync.dma_start(out=wa[:, ST:], in_=w[t, :, ST:KH * D])
                    nc.scalar.dma_start(
                        out=wb[:, ST:], in_=w[t, :, KH * D + ST:])
                    wtiles[t] = (wa, wb)
                elif t == LAST:
                    wla = cp.tile([P, KH * D], WDT, tag="wla")
                    wlb = cp.tile([P, KH * D], WDT, tag="wlb")
                    nc.sync.dma_start(out=wla, in_=w[t, :, : KH * D])
                    nc.scalar.dma_start(out=wlb, in_=w[t, :, KH * D:])
                    wtiles[t] = (wla, wlb)
                else:
                    # two 1MB half-tiles, one per HWDGE ring; alternate
                    # ring assignment per tile so slot-release skew
                    # doesn't pile up on one ring
                    wa = wp.tile([P, KH * D], WDT, tag="wa")
                    wb = wp.tile([P, KH * D], WDT, tag="wb")
                    ea, eb = (nc.sync, nc.scalar) if t % 2 == 0 else (
                        nc.scalar, nc.sync)
                    ea.dma_start(out=wa, in_=w[t, :, : KH * D])
                    eb.dma_start(out=wb, in_=w[t, :, KH * D:])
                    wtiles[t] = (wa, wb)

            bmt = cp.tile([P, NT * 2 * KI], F32, tag="bm")
            nc.gpsimd.dma_start(out=bmt, in_=bm[:])

            # [1,1] identity for PE row->column transposes
            idf = cp.tile([1, 1], ZDT, tag="idf")
            nc.vector.memset(idf, 1.0)

            h = [xt[:, b * KI:(b + 1) * KI] for b in range(BC)]

            def mm_group(t, prow, jb):
                # weight halves are jb-split: wtiles[t][jb][:, ki*512:...]
                cur = h[t % BC]
                wh = wtiles[t][jb]
                for ki in range(KI):
                    nc.tensor.matmul(
                        prow[0:1, jb * 512:(jb + 1) * 512],
                        lhsT=cur[:, ki:ki + 1],
                        rhs=wh[:, ki * 512:(ki + 1) * 512],
                        start=(ki == 0),
                        stop=(ki == KI - 1),
                    )

            def transpose_half(zrow, pcol, jb):
                # pcol is [P, 2*KI] bf16; write even columns only so each
                # PE transpose output lands 4-byte aligned in PSUM.
                for c in range(4):
                    col = 2 * (jb * 4 + c)
                    nc.tensor.transpose(
                        pcol[:, col:col + 1],
                        zrow[0:1, c * 128:(c + 1) * 128],
                        idf,
                    )

            def finish_cols(t, pcol, final):
                # bias + masked relu in column layout:
                #   zb = z + bias;  h = zb - mask * min(zb, 0)
                bias_ap = bmt[:, t * 2 * KI: t * 2 * KI + KI]
                mask_ap = bmt[:, t * 2 * KI + KI: (t + 1) * 2 * KI]
                zcols = pcol.rearrange("p (k two) -> p k two", two=2)[:, :, 0]
                zb = hcp.tile([P, KI], F32, tag="zb")
                nc.vector.tensor_add(out=zb, in0=zcols, in1=bias_ap)
                tmp = hcp.tile([P, KI], F32, tag="tmp")
                nc.vector.scalar_tensor_tensor(
                    out=tmp,
                    in0=zb,
                    scalar=0.0,
                    in1=mask_ap,
                    op0=mybir.AluOpType.min,
                    op1=mybir.AluOpType.mult,
                )
                if final:
                    b = t % BC
                    ho = hcp.tile([P, KI], F32, tag="ho")
                    nc.vector.tensor_sub(out=ho, in0=zb, in1=tmp)
                    eng = nc.scalar if t == NT - 1 else nc.sync
                    eng.dma_start(
                        out=out[:, b * KI:(b + 1) * KI], in_=ho)
                else:
                    hnew = hcp.tile([P, KI], WDT, tag="h")
                    nc.vector.tensor_sub(out=hnew, in0=zb, in1=tmp)
                    h[t % BC] = hnew

            # Processing order: tile 15 (prefetched into its dedicated
            # buffer) runs BEFORE tile 14 (the last streamed tile), so the
            # PE has work while tile 14's weights are still arriving.
            seq = list(range(NT - BC)) + [NT - 1, NT - 2]
            # pending = (t, zrow1, pcol): jb1 transposes + bias/relu of a
            # non-final tile, deferred into the next tile's MM stream.
            pending = None
            for t in seq:
                final = (t >= NT - BC)   # last layer tiles
                prow = psr.tile([1, D], F32)

                if pending is not None and pending[0] % BC == t % BC:
                    # pending tile feeds THIS tile's h: flush its whole
                    # chain before the matmul groups (happens only at the
                    # reordered tail, tile 13 -> 15).
                    pt, pz1, ppc = pending
                    transpose_half(pz1, ppc, 1)
                    finish_cols(pt, ppc, final=False)
                    pending = None

                mm_group(t, prow, 0)

                if pending is not None:
                    pt, pz1, ppc = pending
                    transpose_half(pz1, ppc, 1)
                    pending = None
                else:
                    pt = None

                zrow0 = hrp.tile([1, 512], ZDT, tag="zr0")
                nc.vector.tensor_copy(out=zrow0, in_=prow[0:1, 0:512])

                if pt is not None:
                    finish_cols(pt, ppc, final=False)

                mm_group(t, prow, 1)

                pcol = psc.tile([P, 2 * KI], ZDT)
                transpose_half(zrow0, pcol, 0)
                zrow1 = hrp.tile([1, 512], ZDT, tag="zr1")
                nc.vector.tensor_copy(out=zrow1, in_=prow[0:1, 512:1024])

                if final:
                    # flush this tile's own chain immediately (tail path)
                    transpose_half(zrow1, pcol, 1)
                    finish_cols(t, pcol, final=True)
                else:
                    pending = (t, zrow1, pcol)

            assert pending is None
    nc.finalize()
    return nc


def _get_nc():
    if "nc" not in _CACHE:
        _CACHE["nc"] = _build()
    return _CACHE["nc"]


def _prep_core_inputs(c, x, weights, biases, masks):
    import ml_dtypes
    b0 = c * BC
    # weights[l, b, i, j], i = ki*128 + p  ->  [t, p, jb*4096 + ki*512 + j']
    # (jb-split halves: each matmul group depends on only one 1MB half)
    wc = weights[:, b0:b0 + BC].reshape(NT, KI, P, 2, 512)
    wc = np.ascontiguousarray(wc.transpose(0, 2, 3, 1, 4)).reshape(
        NT, P, KI * D)
    wc = wc.astype(ml_dtypes.bfloat16)
    # x[b, c*128+p] -> [p, b*KI + c]
    xc = x[b0:b0 + BC].reshape(BC, KI, P)
    xc = np.ascontiguousarray(xc.transpose(2, 0, 1)).reshape(P, BC * KI)
    xc = xc.astype(ml_dtypes.bfloat16)
    # bias/mask [l, b, c*128+p] -> [p, (t, {bias,mask}, c)]
    bc = biases[:, b0:b0 + BC].reshape(L, BC, KI, P).transpose(3, 0, 1, 2)
    mc = masks[:, b0:b0 + BC].astype(np.float32).reshape(L, BC, KI, P)
    mc = mc.transpose(3, 0, 1, 2)
    bmc = np.stack([bc, mc], axis=3)  # [p, L, BC, 2, KI]
    bmc = np.ascontiguousarray(bmc).reshape(P, NT * 2 * KI)
    return {"w": wc, "x": xc, "bm": bmc}


def _run(inputs: dict, trace: bool = False, trace_cores=None):
    x = np.asarray(inputs["x"], dtype=np.float32)
    weights = np.asarray(inputs["weights"], dtype=np.float32)
    biases = np.asarray(inputs["biases"], dtype=np.float32)
    masks = np.asarray(inputs["masks"])

    nc = _get_nc()
    in_maps = [
        _prep_core_inputs(c, x, weights, biases, masks) for c in range(NCORES)
    ]
    kw = {}
    if trace_cores is not None:
        kw["trace_cores"] = trace_cores
    res = run_bass_kernel_spmd(
        nc, in_maps, core_ids=list(range(NCORES)), trace=trace, **kw
    )
    outs = []
    for c in range(NCORES):
        oc = res.results[c]["out"]  # [P, BC*KI] column layout
        # out[b, c*128+p] = oc[p, b*KI + c]
        oc = oc.reshape(P, BC, KI).transpose(1, 2, 0).reshape(BC, D)
        outs.append(oc)
    full = np.concatenate(outs, axis=0).astype(np.float32)
    return full, res


def kernel(**inputs) -> np.ndarray:
    full, _ = _run(inputs, trace=False)
    return full


# revision 21
# speedup vs baseline: 1.0028x; 1.0028x over previous
"""Trainium2 Bass kernel for NeuralDisCoCirc forward pass.

Problem: L=8 sequential layers; each layer, per sample b:
    z = h @ W[l,b] + bias[l,b];  h = where(mask[l,b], relu(z), z)
Shapes: x [16,1024] f32, weights [8,16,1024,1024] f32,
        biases/masks [8,16,1024].

Strategy (data-parallel over batch, 2 samples per core, 8 cores):
  - Weights stream as bf16 (host cast): 32 MB per core instead of 64,
    halving the HBM roofline.  rel err ~4e-3 vs the 2e-2 gate.
  - Host lays each core's weight shard out as [t=l*2+b, p, ki*1024+j]
    with W row i = ki*128+p (chunk-major), so per-layer weight loads are
    contiguous [128 x 4KB] DMAs (1 MB halves) on both HWDGE rings with
    4-deep prefetch.  Last tile is re-laid jb-major and streamed as 8
    blocks so its matmuls overlap the DMA tail.
  - h lives in chunk-major column layout ([p, c], element i = c*128+p)
    and is the matmul stationary operand.  Per layer: 2 PSUM groups of
    8 accumulating matmuls (h chunk [128,1] stationary, W chunk
    [128,512] bf16 moving) produce z as a [1,1024] PSUM row.
  - The row->column transpose runs on the PE (8 transposes of [1,128]
    via identity, ~0.1us each) instead of a SWDGE scatter: the scatter's
    ~4-6us software-DMA latency was the old bottleneck (PE idled 4.7us
    per tile waiting for h; DMA rings stalled on slot release).
    Pipelined order per tile t:
      [MM(t) jb0 x8][T(t-1) jb1 x4][MM(t) jb1 x8][T(t) jb0 x4]
    so every transpose has its PSUM->SBUF copy latency hidden behind a
    matmul group, and h(t-1) is ready one full tile before it is needed.
  - DVE does the two PSUM-row->SBUF copies (cast to bf16) and the
    bias + masked-relu (h = zb - mask*min(zb,0)) on [128,8] columns.
  - Final layer skips the column layout: bias + masked relu run in ROW
    layout per jb half and the output ships row-contiguous.
"""

import numpy as np

import concourse.bass as bass
import concourse.mybir as mybir
from concourse import bacc
from concourse.tile import TileContext
from concourse.bass_utils import run_bass_kernel_spmd

L = 8          # layers
B = 16         # full batch
D = 1024       # width
NCORES = 8
BC = B // NCORES   # samples per core (2)
NT = L * BC        # (layer, sample) tiles per core (16)
KI = D // 128      # 8 chunks of 128 along the contraction dim
P = 128

F32 = mybir.dt.float32
BF16 = mybir.dt.bfloat16

WDT = BF16   # weight / h dtype on device
ZDT = BF16   # zrow dtype for the PE transposes (bf16: 1 cyc/row)

_CACHE = {}


def _build() -> bass.Bass:
    nc = bacc.Bacc("TRN2", target_bir_lowering=False, debug=False)
    w = nc.declare_dram_parameter("w", [NT, P, KI * D], WDT, isOutput=False)
    x = nc.declare_dram_parameter("x", [P, BC * KI], WDT, isOutput=False)
    bm = nc.declare_dram_parameter("bm", [P, NT * 2 * KI], F32, isOutput=False)
    # output ships in column layout ([p, b*KI + c], element i = c*128+p);
    # the host transposes it back to row layout.
    out = nc.declare_dram_parameter("out", [P, BC * KI], F32, isOutput=True)

    with TileContext(nc) as tc:
        with (
            tc.tile_pool(name="wp", bufs=8) as wp,  # per-tag: 8 x 1MB x 2 tags
            tc.tile_pool(name="const", bufs=1) as cp,
            tc.tile_pool(name="hrow", bufs=4) as hrp,
            tc.tile_pool(name="hcol", bufs=4) as hcp,
            tc.tile_pool(name="psr", bufs=2, space="PSUM") as psr,
            tc.tile_pool(name="psc", bufs=2, space="PSUM") as psc,
        ):
            # x first on the sync HWDGE ring: tiny (4KB) and it gates the
            # very first matmul, so it must land before the weight stream.
            xt = cp.tile([P, BC * KI], WDT, tag="x")
            nc.sync.dma_start(out=xt, in_=x[:])

            KH = KI // 2  # ki chunks per half-tile
            LAST = NT - 1
            ST = 512      # starter block columns for tile 0 (fast first MM)
            wtiles = {}
            # Issue order: tiles 0..5 (the ungated prefix at bufs=6), then
            # the LAST tile into its own dedicated buffer (so its stream
            # is not serialized by wa/wb slot recycling at the tail),
            # then tiles 6..14.
            issue_order = list(range(8)) + [LAST] + list(range(8, LAST))
            for t in issue_order:
                if t == 0:
                    wa = wp.tile([P, KH * D], WDT, tag="wa")
                    wb = wp.tile([P, KH * D], WDT, tag="wb")
                    nc.sync.dma_start(out=wa[:, :ST], in_=w[t, :, :ST])
                    nc.scalar.dma_start(
                        out=wb[:, :ST], in_=w[t, :, KH * D:KH * D + ST])
                    nc.sync.dma_start(out=wa[:, ST:], in_=w[t, :, ST:KH * D])
                    nc.scalar.dma_start(
                        out=wb[:, ST:], in_=w[t, :, KH * D + ST:])
                    wtiles[t] = (wa, wb)
                elif t == LAST:
                    wla = cp.tile([P, KH * D], WDT, tag="wla")
                    wlb = cp.tile([P, KH * D], WDT, tag="wlb")
                    nc.sync.dma_start(out=wla, in_=w[t, :, : KH * D])
                    nc.scalar.dma_start(out=wlb, in_=w[t, :, KH * D:])
                    wtiles[t] = (wla, wlb)
                else:
                    # two 1MB half-tiles, one per HWDGE ring; alternate
                    # ring assignment per tile so slot-release skew
                    # doesn't pile up on one ring
                    wa = wp.tile([P, KH * D], WDT, tag="wa")
                    wb = wp.tile([P, KH * D], WDT, tag="wb")
                    ea, eb = (nc.sync, nc.scalar) if t % 2 == 0 else (
                        nc.scalar, nc.sync)
                    ea.dma_start(out=wa, in_=w[t, :, : KH * D])
                    eb.dma_start(out=wb, in_=w[t, :, KH * D:])
                    wtiles[t] = (wa, wb)

            bmt = cp.tile([P, NT * 2 * KI], F32, tag="bm")
            nc.gpsimd.dma_start(out=bmt, in_=bm[:])

            # [1,1] identity for PE row->column transposes
            idf = cp.tile([1, 1], ZDT, tag="idf")
            nc.vector.memset(idf, 1.0)

            h = [xt[:, b * KI:(b + 1) * KI] for b in range(BC)]

            def mm_group(t, prow, jb):
                # weight halves are jb-split: wtiles[t][jb][:, ki*512:...]
                cur = h[t % BC]
                wh = wtiles[t][jb]
                for ki in range(KI):
                    nc.tensor.matmul(
                        prow[0:1, jb * 512:(jb + 1) * 512],
                        lhsT=cur[:, ki:ki + 1],
                        rhs=wh[:, ki * 512:(ki + 1) * 512],
                        start=(ki == 0),
                        stop=(ki == KI - 1),
                    )

            def transpose_half(zrow, pcol, jb):
                # pcol is [P, 2*KI] bf16; write even columns only so each
                # PE transpose output lands 4-byte aligned in PSUM.
                for c in range(4):
                    col = 2 * (jb * 4 + c)
                    nc.tensor.transpose(
                        pcol[:, col:col + 1],
                        zrow[0:1, c * 128:(c + 1) * 128],
                        idf,
                    )

            def finish_cols(t, pcol, final):
                # bias + masked relu in column layout:
                #   zb = z + bias;  h = zb - mask * min(zb, 0)
                bias_ap = bmt[:, t * 2 * KI: t * 2 * KI + KI]
                mask_ap = bmt[:, t * 2 * KI + KI: (t + 1) * 2 * KI]
                zcols = pcol.rearrange("p (k two) -> p k two", two=2)[:, :, 0]
                zb = hcp.tile([P, KI], F32, tag="zb")
                nc.vector.tensor_add(out=zb, in0=zcols, in1=bias_ap)
                tmp = hcp.tile([P, KI], F32, tag="tmp")
                nc.vector.scalar_tensor_tensor(
                    out=tmp,
                    in0=zb,
                    scalar=0.0,
                    in1=mask_ap,
                    op0=mybir.AluOpType.min,
                    op1=mybir.AluOpType.mult,
                )
                if final:
                    b = t % BC
                    ho = hcp.tile([P, KI], F32, tag="ho")
                    nc.vector.tensor_sub(out=ho, in0=zb, in1=tmp)
                    eng = nc.scalar if t == NT - 1 else nc.sync
                    eng.dma_start(
                        out=out[:, b * KI:(b + 1) * KI], in_=ho)
                else:
                    hnew = hcp.tile([P, KI], WDT, tag="h")
                    nc.vector.tensor_sub(out=hnew, in0=zb, in1=tmp)
                    h[t % BC] = hnew

            # Processing order: tile 15 (prefetched into its dedicated
            # buffer) runs BEFORE tile 14 (the last streamed tile), so the
            # PE has work while tile 14's weights are still arriving.
            seq = list(range(NT - BC)) + [NT - 1, NT - 2]
            # pending = (t, zrow1, pcol): jb1 transposes + bias/relu of a
            # non-final tile, deferred into the next tile's MM stream.
            pending = None
            for t in seq:
                final = (t >= NT - BC)   # last layer tiles
                prow = psr.tile([1, D], F32)

                if pending is not None and pending[0] % BC == t % BC:
                    # pending tile feeds THIS tile's h: flush its whole
                    # chain before the matmul groups (happens only at the
                    # reordered tail, tile 13 -> 15).
                    pt, pz1, ppc = pending
                    transpose_half(pz1, ppc, 1)
                    finish_cols(pt, ppc, final=False)
                    pending = None

                mm_group(t, prow, 0)

                if pending is not None:
                    pt, pz1, ppc = pending
                    transpose_half(pz1, ppc, 1)
                    pending = None
                else:
                    pt = None

                zrow0 = hrp.tile([1, 512], ZDT, tag="zr0")
                nc.vector.tensor_copy(out=zrow0, in_=prow[0:1, 0:512])

                if pt is not None:
                    finish_cols(pt, ppc, final=False)

                mm_group(t, prow, 1)

                pcol = psc.tile([P, 2 * KI], ZDT)
                transpose_half(zrow0, pcol, 0)
                zrow1 = hrp.tile([1, 512], ZDT, tag="zr1")
                nc.vector.tensor_copy(out=zrow1, in_=prow[0:1, 512:1024])

                if final:
                    # flush this tile's own chain immediately (tail path)
                    transpose_half(zrow1, pcol, 1)
                    finish_cols(t, pcol, final=True)
                else:
                    pending = (t, zrow1, pcol)

            assert pending is None
    nc.finalize()
    return nc


def _get_nc():
    if "nc" not in _CACHE:
        _CACHE["nc"] = _build()
    return _CACHE["nc"]


def _prep_core_inputs(c, x, weights, biases, masks):
    import ml_dtypes
    b0 = c * BC
    # weights[l, b, i, j], i = ki*128 + p  ->  [t, p, jb*4096 + ki*512 + j']
    # (jb-split halves: each matmul group depends on only one 1MB half)
    wc = weights[:, b0:b0 + BC].reshape(NT, KI, P, 2, 512)
    wc = np.ascontiguousarray(wc.transpose(0, 2, 3, 1, 4)).reshape(
        NT, P, KI * D)
    wc = wc.astype(ml_dtypes.bfloat16)
    # x[b, c*128+p] -> [p, b*KI + c]
    xc = x[b0:b0 + BC].reshape(BC, KI, P)
    xc = np.ascontiguousarray(xc.transpose(2, 0, 1)).reshape(P, BC * KI)
    xc = xc.astype(ml_dtypes.bfloat16)
    # bias/mask [l, b, c*128+p] -> [p, (t, {bias,mask}, c)]
    bc = biases[:, b0:b0 + BC].reshape(L, BC, KI, P).transpose(3, 0, 1, 2)
    mc = masks[:, b0:b0 + BC].astype(np.float32).reshape(L, BC, KI, P)
    mc = mc.transpose(3, 0, 1, 2)
    bmc = np.stack([bc, mc], axis=3)  # [p, L, BC, 2, KI]
    bmc = np.ascontiguousarray(bmc).reshape(P, NT * 2 * KI)
    return {"w": wc, "x": xc, "bm": bmc}


def _run(inputs: dict, trace: bool = False, trace_cores=None):
    x = np.asarray(inputs["x"], dtype=np.float32)
    weights = np.asarray(inputs["weights"], dtype=np.float32)
    biases = np.asarray(inputs["biases"], dtype=np.float32)
    masks = np.asarray(inputs["masks"])

    nc = _get_nc()
    in_maps = [
        _prep_core_inputs(c, x, weights, biases, masks) for c in range(NCORES)
    ]
    kw = {}
    if trace_cores is not None:
        kw["trace_cores"] = trace_cores
    res = run_bass_kernel_spmd(
        nc, in_maps, core_ids=list(range(NCORES)), trace=trace, **kw
    )
    outs = []
    for c in range(NCORES):
        oc = res.results[c]["out"]  # [P, BC*KI] column layout
        # out[b, c*128+p] = oc[p, b*KI + c]
        oc = oc.reshape(P, BC, KI).transpose(1, 2, 0).reshape(BC, D)
        outs.append(oc)
    full = np.concatenate(outs, axis=0).astype(np.float32)
    return full, res


def kernel(**inputs) -> np.ndarray:
    full, _ = _run(inputs, trace=False)
    return full


# revision 23
# speedup vs baseline: 1.1068x; 1.1037x over previous
"""Trainium2 Bass kernel for NeuralDisCoCirc forward pass.

Problem: L=8 sequential layers; each layer, per sample b:
    z = h @ W[l,b] + bias[l,b];  h = where(mask[l,b], relu(z), z)
Shapes: x [16,1024] f32, weights [8,16,1024,1024] f32,
        biases/masks [8,16,1024].

Strategy (data-parallel over batch, 2 samples per core, 8 cores):
  - Weights stream as bf16 (host cast): 32 MB per core instead of 64,
    halving the HBM roofline.  rel err ~4e-3 vs the 2e-2 gate.
  - Host lays each core's weight shard out as [t=l*2+b, p, ki*1024+j]
    with W row i = ki*128+p (chunk-major), so per-layer weight loads are
    contiguous [128 x 4KB] DMAs (1 MB halves) on both HWDGE rings with
    4-deep prefetch.  Last tile is re-laid jb-major and streamed as 8
    blocks so its matmuls overlap the DMA tail.
  - h lives in chunk-major column layout ([p, c], element i = c*128+p)
    and is the matmul stationary operand.  Per layer: 2 PSUM groups of
    8 accumulating matmuls (h chunk [128,1] stationary, W chunk
    [128,512] bf16 moving) produce z as a [1,1024] PSUM row.
  - The row->column transpose runs on the PE (8 transposes of [1,128]
    via identity, ~0.1us each) instead of a SWDGE scatter: the scatter's
    ~4-6us software-DMA latency was the old bottleneck (PE idled 4.7us
    per tile waiting for h; DMA rings stalled on slot release).
    Pipelined order per tile t:
      [MM(t) jb0 x8][T(t-1) jb1 x4][MM(t) jb1 x8][T(t) jb0 x4]
    so every transpose has its PSUM->SBUF copy latency hidden behind a
    matmul group, and h(t-1) is ready one full tile before it is needed.
  - DVE does the two PSUM-row->SBUF copies (cast to bf16) and the
    bias + masked-relu (h = zb - mask*min(zb,0)) on [128,8] columns.
  - Final layer skips the column layout: bias + masked relu run in ROW
    layout per jb half and the output ships row-contiguous.
"""

import numpy as np

import concourse.bass as bass
import concourse.mybir as mybir
from concourse import bacc
from concourse.tile import TileContext
from concourse.bass_utils import run_bass_kernel_spmd

L = 8          # layers
B = 16         # full batch
D = 1024       # width
NCORES = 8
BC = B // NCORES   # samples per core (2)
NT = L * BC        # (layer, sample) tiles per core (16)
KI = D // 128      # 8 chunks of 128 along the contraction dim
P = 128

F32 = mybir.dt.float32
BF16 = mybir.dt.bfloat16

WDT = BF16   # weight / h dtype on device
ZDT = BF16   # zrow dtype for the PE transposes (bf16: 1 cyc/row)

_CACHE = {}


def _build() -> bass.Bass:
    nc = bacc.Bacc("TRN2", target_bir_lowering=False, debug=False)
    w = nc.declare_dram_parameter("w", [NT, P, KI * D], WDT, isOutput=False)
    x = nc.declare_dram_parameter("x", [P, BC * KI], WDT, isOutput=False)
    bm = nc.declare_dram_parameter("bm", [P, NT * 2 * KI], F32, isOutput=False)
    # output ships in column layout ([p, b*KI + c], element i = c*128+p);
    # the host transposes it back to row layout.
    out = nc.declare_dram_parameter("out", [P, BC * KI], F32, isOutput=True)

    with TileContext(nc) as tc:
        with (
            tc.tile_pool(name="wp", bufs=6) as wp,  # per-tag: 6 x 1MB x 2 tags
            tc.tile_pool(name="const", bufs=1) as cp,
            tc.tile_pool(name="hrow", bufs=4) as hrp,
            tc.tile_pool(name="hcol", bufs=4) as hcp,
            tc.tile_pool(name="psr", bufs=2, space="PSUM") as psr,
            tc.tile_pool(name="psc", bufs=2, space="PSUM") as psc,
        ):
            # x first on the sync HWDGE ring: tiny (4KB) and it gates the
            # very first matmul, so it must land before the weight stream.
            xt = cp.tile([P, BC * KI], WDT, tag="x")
            nc.sync.dma_start(out=xt, in_=x[:])

            KH = KI // 2  # ki chunks per half-tile
            LAST = NT - 1
            ST = 512      # starter block columns for tile 0 (fast first MM)
            wtiles = {}
            # Issue order: tiles 0..5 (the ungated prefix at bufs=6), then
            # the LAST tile into its own dedicated buffer (so its stream
            # is not serialized by wa/wb slot recycling at the tail),
            # then tiles 6..14.
            issue_order = list(range(6)) + [LAST] + list(range(6, LAST))
            for t in issue_order:
                if t == 0:
                    wa = wp.tile([P, KH * D], WDT, tag="wa")
                    wb = wp.tile([P, KH * D], WDT, tag="wb")
                    nc.sync.dma_start(out=wa[:, :ST], in_=w[t, :, :ST])
                    nc.scalar.dma_start(
                        out=wb[:, :ST], in_=w[t, :, KH * D:KH * D + ST])
                    nc.sync.dma_start(out=wa[:, ST:], in_=w[t, :, ST:KH * D])
                    nc.scalar.dma_start(
                        out=wb[:, ST:], in_=w[t, :, KH * D + ST:])
                    wtiles[t] = (wa, wb)
                elif t == LAST:
                    wla = cp.tile([P, KH * D], WDT, tag="wla")
                    wlb = cp.tile([P, KH * D], WDT, tag="wlb")
                    nc.sync.dma_start(out=wla, in_=w[t, :, : KH * D])
                    nc.scalar.dma_start(out=wlb, in_=w[t, :, KH * D:])
                    wtiles[t] = (wla, wlb)
                else:
                    # two 1MB half-tiles, one per HWDGE ring; alternate
                    # ring assignment per tile so slot-release skew
                    # doesn't pile up on one ring
                    wa = wp.tile([P, KH * D], WDT, tag="wa")
                    wb = wp.tile([P, KH * D], WDT, tag="wb")
                    ea, eb = (nc.sync, nc.scalar) if t % 2 == 0 else (
                        nc.scalar, nc.sync)
                    ea.dma_start(out=wa, in_=w[t, :, : KH * D])
                    eb.dma_start(out=wb, in_=w[t, :, KH * D:])
                    wtiles[t] = (wa, wb)

            bmt = cp.tile([P, NT * 2 * KI], F32, tag="bm")
            nc.gpsimd.dma_start(out=bmt, in_=bm[:])

            # [1,1] identity for PE row->column transposes
            idf = cp.tile([1, 1], ZDT, tag="idf")
            nc.vector.memset(idf, 1.0)

            h = [xt[:, b * KI:(b + 1) * KI] for b in range(BC)]

            def mm_group(t, prow, jb):
                # weight halves are jb-split: wtiles[t][jb][:, ki*512:...]
                cur = h[t % BC]
                wh = wtiles[t][jb]
                for ki in range(KI):
                    nc.tensor.matmul(
                        prow[0:1, jb * 512:(jb + 1) * 512],
                        lhsT=cur[:, ki:ki + 1],
                        rhs=wh[:, ki * 512:(ki + 1) * 512],
                        start=(ki == 0),
                        stop=(ki == KI - 1),
                    )

            def transpose_half(zrow, pcol, jb):
                # pcol is [P, 2*KI] bf16; write even columns only so each
                # PE transpose output lands 4-byte aligned in PSUM.
                for c in range(4):
                    col = 2 * (jb * 4 + c)
                    nc.tensor.transpose(
                        pcol[:, col:col + 1],
                        zrow[0:1, c * 128:(c + 1) * 128],
                        idf,
                    )

            def finish_cols(t, pcol, final):
                # bias + masked relu in column layout:
                #   zb = z + bias;  h = zb - mask * min(zb, 0)
                bias_ap = bmt[:, t * 2 * KI: t * 2 * KI + KI]
                mask_ap = bmt[:, t * 2 * KI + KI: (t + 1) * 2 * KI]
                zcols = pcol.rearrange("p (k two) -> p k two", two=2)[:, :, 0]
                zb = hcp.tile([P, KI], F32, tag="zb")
                nc.vector.tensor_add(out=zb, in0=zcols, in1=bias_ap)
                tmp = hcp.tile([P, KI], F32, tag="tmp")
                nc.vector.scalar_tensor_tensor(
                    out=tmp,
                    in0=zb,
                    scalar=0.0,
                    in1=mask_ap,
                    op0=mybir.AluOpType.min,
                    op1=mybir.AluOpType.mult,
                )
                if final:
                    b = t % BC
                    ho = hcp.tile([P, KI], F32, tag="ho")
                    nc.vector.tensor_sub(out=ho, in0=zb, in1=tmp)
                    eng = nc.scalar if t == NT - 1 else nc.sync
                    eng.dma_start(
                        out=out[:, b * KI:(b + 1) * KI], in_=ho)
                else:
                    hnew = hcp.tile([P, KI], WDT, tag="h")
                    nc.vector.tensor_sub(out=hnew, in0=zb, in1=tmp)
                    h[t % BC] = hnew

            seq = list(range(NT))
            # pending = (t, zrow1, pcol): jb1 transposes + bias/relu of a
            # non-final tile, deferred into the next tile's MM stream.
            pending = None
            for t in seq:
                final = (t >= NT - BC)   # last layer tiles
                prow = psr.tile([1, D], F32)

                if pending is not None and pending[0] % BC == t % BC:
                    # pending tile feeds THIS tile's h: flush its whole
                    # chain before the matmul groups (happens only at the
                    # reordered tail, tile 13 -> 15).
                    pt, pz1, ppc = pending
                    transpose_half(pz1, ppc, 1)
                    finish_cols(pt, ppc, final=False)
                    pending = None

                mm_group(t, prow, 0)

                if pending is not None:
                    pt, pz1, ppc = pending
                    transpose_half(pz1, ppc, 1)
                    pending = None
                else:
                    pt = None

                zrow0 = hrp.tile([1, 512], ZDT, tag="zr0")
                nc.vector.tensor_copy(out=zrow0, in_=prow[0:1, 0:512])

                if pt is not None:
                    finish_cols(pt, ppc, final=False)

                mm_group(t, prow, 1)

                pcol = psc.tile([P, 2 * KI], ZDT)
                transpose_half(zrow0, pcol, 0)
                zrow1 = hrp.tile([1, 512], ZDT, tag="zr1")
                nc.vector.tensor_copy(out=zrow1, in_=prow[0:1, 512:1024])

                if final:
                    # flush this tile's own chain immediately (tail path)
                    transpose_half(zrow1, pcol, 1)
                    finish_cols(t, pcol, final=True)
                else:
                    pending = (t, zrow1, pcol)

            assert pending is None
    nc.finalize()
    return nc


def _get_nc():
    if "nc" not in _CACHE:
        _CACHE["nc"] = _build()
    return _CACHE["nc"]


def _prep_core_inputs(c, x, weights, biases, masks):
    import ml_dtypes
    b0 = c * BC
    # weights[l, b, i, j], i = ki*128 + p  ->  [t, p, jb*4096 + ki*512 + j']
    # (jb-split halves: each matmul group depends on only one 1MB half)
    wc = weights[:, b0:b0 + BC].reshape(NT, KI, P, 2, 512)
    wc = np.ascontiguousarray(wc.transpose(0, 2, 3, 1, 4)).reshape(
        NT, P, KI * D)
    wc = wc.astype(ml_dtypes.bfloat16)
    # x[b, c*128+p] -> [p, b*KI + c]
    xc = x[b0:b0 + BC].reshape(BC, KI, P)
    xc = np.ascontiguousarray(xc.transpose(2, 0, 1)).reshape(P, BC * KI)
    xc = xc.astype(ml_dtypes.bfloat16)
    # bias/mask [l, b, c*128+p] -> [p, (t, {bias,mask}, c)]
    bc = biases[:, b0:b0 + BC].reshape(L, BC, KI, P).transpose(3, 0, 1, 2)
    mc = masks[:, b0:b0 + BC].astype(np.float32).reshape(L, BC, KI, P)
    mc = mc.transpose(3, 0, 1, 2)
    bmc = np.stack([bc, mc], axis=3)  # [p, L, BC, 2, KI]
    bmc = np.ascontiguousarray(bmc).reshape(P, NT * 2 * KI)
    return {"w": wc, "x": xc, "bm": bmc}


def _run(inputs: dict, trace: bool = False, trace_cores=None):
    x = np.asarray(inputs["x"], dtype=np.float32)
    weights = np.asarray(inputs["weights"], dtype=np.float32)
    biases = np.asarray(inputs["biases"], dtype=np.float32)
    masks = np.asarray(inputs["masks"])

    nc = _get_nc()
    in_maps = [
        _prep_core_inputs(c, x, weights, biases, masks) for c in range(NCORES)
    ]
    kw = {}
    if trace_cores is not None:
        kw["trace_cores"] = trace_cores
    res = run_bass_kernel_spmd(
        nc, in_maps, core_ids=list(range(NCORES)), trace=trace, **kw
    )
    outs = []
    for c in range(NCORES):
        oc = res.results[c]["out"]  # [P, BC*KI] column layout
        # out[b, c*128+p] = oc[p, b*KI + c]
        oc = oc.reshape(P, BC, KI).transpose(1, 2, 0).reshape(BC, D)
        outs.append(oc)
    full = np.concatenate(outs, axis=0).astype(np.float32)
    return full, res


def kernel(**inputs) -> np.ndarray:
    full, _ = _run(inputs, trace=False)
    return full
